# revision 35
# baseline (speedup 1.0000x reference)
"""DGPLoss Trainium2 kernel.

Reference computation (see problem):
  - split [B,C,H,W] into non-overlapping 5x5 patches (510x510 used of 512)
  - seg_sq(p) = sum_c (seg[c, center(p)] - seg[c, p])^2        (C=64)
  - dep_diff(p) = |dep[center(p)] - dep[p]|
  - loss = exp(-dep_diff/10 - seg_sq)
  - mask = (dep_diff > 1e-8) & (sqrt(seg_sq) > 1e-8) & (dep > 1e-8)
    (center pixels fall out automatically: dep_diff == 0 exactly there)
  - out = sum(loss*mask) / max(sum(mask), 1)

Sharding: 8 cores = 4 batches x 2 row-halves (255 rows = 51 strips each).
Each core returns [128, 2] partials (per-psum-partition loss-sum and count);
the host sums and divides.

Per-core layout: pixel-row PAIRS (q, q+128) live on SBUF partitions as
[64ch | 64ch] -- consecutive DRAM rows per partition half, so x-tile DMAs
move 8KB-contiguous runs. DVE subtracts a per-pair patch-center tile
(host-staged, broadcast along the free axis with stride-0 APs), ACT squares
into float32r, and the PE reduces channels with sliding-window ones-block
matmuls (M=128, float32r, full-rate) accumulating 64 matmuls per PSUM fill,
so PSUM partition p of fill f holds pixel row 64f + p//2 + 128*(p%2).
The dep branch uses host-staged center tables; masks/exp/reductions run on
[128, 510] pixel tiles. A post-pass splits semaphore waits onto
EventSemaphore carriers (walrus allows one sync wait per instruction).
"""

import os
import numpy as np
from contextlib import ExitStack

C = 64          # channels
ROWS = 255      # pixel rows per core
W = 510         # used width
NG = 102        # patch groups along w
NTILES = 32     # x tiles (4 row-pairs each)
NPAIRS = 128    # row pairs (q, q+128), incl. dup tail
CHUNK = 17      # center-tile keys per staged chunk

_NC_CACHE = {}


def _row_of(f, p):
    """Global pixel row held by PSUM fill f, partition p (dup row clamped)."""
    r = 64 * f + p // 2 + 128 * (p % 2)
    return min(r, 254)


def _center_keys():
    """Distinct (strip of row q, strip of row q+128) pairs in first-use
    order. Shared by the host shard builder and the kernel builder."""
    keys, index = [], {}
    for q in range(NPAIRS):
        k = (q // 5, min(q + 128, 254) // 5)
        if k not in index:
            index[k] = len(keys)
            keys.append(k)
    return keys, index


_KEYS, _KIDX = _center_keys()
NKEYS = len(_KEYS)          # 51

# walrus codegen in this toolchain allows only ONE sync-wait command on most
# engine instruction structs ("Too many sync wait commands"). Tile's scheduler
# freely emits several. Split the excess onto preceding same-engine
# EventSemaphore instructions (pure wait carriers) -- semantically identical:
# the engine blocks at the same program point either way.
_WAIT_LIMITS = {"ISA": 99}  # raw-encoded payload; do not touch


def _split_excess_waits(nc):
    import json
    import bass_rust

    mj = json.loads(bass_rust.module_to_json_bytes(nc.m))
    n_split = 0
    for fn in mj["functions"]:
        for blk in fn["blocks"]:
            out = []
            for inst in blk["instructions"]:
                si = inst.get("sync_info") or {}
                waits = si.get("on_wait") or []
                limit = _WAIT_LIMITS.get(inst["opcode"], 1)
                if len(waits) > limit:
                    extra, keep = waits[:-limit], waits[-limit:]
                    for i, w in enumerate(extra):
                        out.append({
                            "debug": inst.get("debug"),
                            "engine": inst["engine"],
                            "ins": [], "outs": [],
                            "name": f"{inst['name']}-xw{i}",
                            "opcode": "EventSemaphore",
                            "sync_info": {"on_update": [], "on_wait": [w]},
                        })
                        n_split += 1
                    si["on_wait"] = keep
                out.append(inst)
            blk["instructions"] = out
    nc.m = bass_rust.module_from_json_bytes(json.dumps(mj).encode())
    return n_split


def _build_module(reduce_dt_name="float32r"):
    import concourse.bass as bass
    import concourse.mybir as mybir
    import concourse.tile as tile

    f32 = mybir.dt.float32
    rdt = getattr(mybir.dt, reduce_dt_name)
    AF = mybir.ActivationFunctionType
    OP = mybir.AluOpType

    nc = bass.Bass()
    seg = nc.declare_dram_parameter("seg", (C, ROWS, 512), f32, isOutput=False)
    dpx = nc.declare_dram_parameter("dpx", (2, 128, W), f32, isOutput=False)
    dctr = nc.declare_dram_parameter("dctr", (2, 128, NG), f32, isOutput=False)
    pb = nc.declare_dram_parameter("pb", (128, 256), rdt, isOutput=False)
    pcc0 = nc.declare_dram_parameter("pcc0", (128, NG), f32, isOutput=False)
    out = nc.declare_dram_parameter("out", (128, 2), f32, isOutput=True)

    with tile.TileContext(nc) as tc, ExitStack() as ctx:
        consts = ctx.enter_context(tc.tile_pool(name="consts", bufs=1))
        xpool = ctx.enter_context(tc.tile_pool(name="x", bufs=5))
        dpool = ctx.enter_context(tc.tile_pool(name="d", bufs=3))
        d2pool = ctx.enter_context(tc.tile_pool(name="d2", bufs=3))
        cspool = ctx.enter_context(tc.tile_pool(name="cs", bufs=6))
        pix = ctx.enter_context(tc.tile_pool(name="pix", bufs=2))
        psum = ctx.enter_context(tc.tile_pool(name="psum", bufs=2, space="PSUM"))

        # PB: sliding-window ones-block. Window j = PB[:, 126-2j : 254-2j] is
        # a [128, 128] lhsT whose only nonzeros put (sum over partitions 0:64)
        # in out row 2j and (sum over 64:128) in out row 2j+1; the other 126
        # out rows accumulate zeros. M=128 keeps every matmul's PSUM dst at
        # partition 0 (this walrus rejects col-tiled PSUM offsets). The
        # reduce dtype (float32r) cannot be memset (walrus ISA check), so it
        # is host-staged.
        PB = consts.tile([128, 256], rdt)
        nc.sync.dma_start(out=PB[:], in_=pb[:, :])

        zero_bias = consts.tile([128, 1], f32)
        nc.vector.memset(zero_bias[:], 0.0)
        # comparison thresholds as [128, 1] const tiles: tensor_scalar lowers
        # to an ISA struct with a single sync-wait slot, so the masks use
        # tensor_tensor + broadcast instead.
        eps_d10 = consts.tile([128, 1], f32)
        nc.vector.memset(eps_d10[:], 1e-9)
        eps_ssq = consts.tile([128, 1], f32)
        nc.vector.memset(eps_ssq[:], 1e-16)
        eps_dep = consts.tile([128, 1], f32)
        nc.vector.memset(eps_dep[:], 1e-8)

        # ---- dep pixel tiles, partition p <-> row _row_of(f, p);
        # host-staged pre-permuted (fill-1 partition 127 zeroed on host) ----
        dep_px0 = consts.tile([128, W], f32, tag="dep_px0")
        nc.sync.dma_start(out=dep_px0[:], in_=dpx[0])
        dep_px1 = consts.tile([128, W], f32, tag="dep_px1")
        nc.sync.dma_start(out=dep_px1[:], in_=dpx[1])

        # dep patch-center tiles (host-staged): dpsd[f][p, g] =
        # dep[center_row(_row_of(f, p)), 5g+2]
        dpsd = []
        for f in range(2):
            t = consts.tile([128, NG], f32, tag=f"dpsd{f}", name=f"dpsd{f}")
            nc.sync.dma_start(out=t[:], in_=dctr[f])
            dpsd.append(t)

        # accumulators
        lacc = [consts.tile([128, 1], f32, tag=f"lacc{f}", name=f"lacc{f}")
                for f in range(2)]
        cacc = [consts.tile([128, 1], f32, tag=f"cacc{f}", name=f"cacc{f}")
                for f in range(2)]

        psum_tiles = [psum.tile([128, W], f32, tag="ps", name=f"ps{_f}",
                                padded_shape=[128, 512]) for _f in range(2)]

        # ---- center tiles: extracted on-device from the x tiles by GPSIMD
        # (strided engine reads are fine; a strided DMA gather was not). In
        # this pairing each key's A-half center row (5sa+2 <= 127) sits on
        # partitions 0:64 of its x tile and the B-half row (5sb+2 >= 132) on
        # partitions 64:128 -- both partition-aligned with the cs tile. The
        # one boundary key (0, 25) (B-half center is row 127, an A-half row)
        # is host-staged via pcc0.
        cs_tiles = {}

        def make_cs(ki):
            sa, sb = _KEYS[ki]
            t = cspool.tile([128, NG], f32, tag="cs", name=f"cs_{sa}_{sb}")
            if (sa, sb) == (0, 25):
                nc.sync.dma_start(out=t[:], in_=pcc0[:, :])
            else:
                ra = 5 * sa + 2
                nc.gpsimd.tensor_copy(t[0:64, :],
                                      xts[ra // 8][0:64, ra % 8, 2:W:5])
                qb = 5 * sb + 2 - 128
                nc.gpsimd.tensor_copy(t[64:128, :],
                                      xts[qb // 8][64:128, qb % 8, 2:W:5])
            cs_tiles[ki] = t

        def get_cs(ki):
            return cs_tiles[ki][:, :]

        # pixel phase is split: the dep-only part can run as soon as the dep
        # tiles land (start of kernel); only the psum-dependent part sits on
        # the critical tail after the fill's last matmul.
        dpre = {}

        def pixel_pre(f):
            dep_px = dep_px0 if f == 0 else dep_px1
            ts = pix.tile([128, NG, 5], f32, tag=f"ts{f}", name=f"ts{f}")
            nc.vector.tensor_tensor(
                ts[:],
                dep_px[:].rearrange("p (g f) -> p g f", f=5),
                dpsd[f][:, :, None].to_broadcast((128, NG, 5)),
                OP.subtract,
            )
            d10 = pix.tile([128, W], f32, tag=f"d10{f}", name=f"d10{f}")
            nc.scalar.activation(d10[:], ts[:].rearrange("p g f -> p (g f)"),
                                 AF.Abs, bias=zero_bias[:], scale=0.1)
            m13 = pix.tile([128, W], f32, tag=f"m13{f}", name=f"m13{f}")
            m1 = pix.tile([128, W], f32, tag=f"m1{f}", name=f"m1{f}")
            nc.vector.tensor_tensor(m1[:], d10[:],
                                    eps_d10[:].to_broadcast((128, W)), OP.is_gt)
            m3 = pix.tile([128, W], f32, tag=f"m3{f}", name=f"m3{f}")
            nc.vector.tensor_tensor(m3[:], dep_px[:],
                                    eps_dep[:].to_broadcast((128, W)), OP.is_gt)
            nc.vector.tensor_tensor(m13[:], m1[:], m3[:], OP.mult)
            dpre[f] = (d10, m13)

        def pixel_finish(f):
            d10, m13 = dpre[f]
            u = pix.tile([128, W], f32, tag="u")
            nc.vector.tensor_tensor(u[:], d10[:], psum_tiles[f][:], OP.add)
            E = pix.tile([128, W], f32, tag="E")
            nc.scalar.activation(E[:], u[:], AF.Exp, bias=zero_bias[:],
                                 scale=-1.0)
            m2 = pix.tile([128, W], f32, tag="m2")
            nc.vector.tensor_tensor(m2[:], psum_tiles[f][:],
                                    eps_ssq[:].to_broadcast((128, W)), OP.is_gt)
            mask = pix.tile([128, W], f32, tag="mask")
            nc.vector.tensor_tensor(mask[:], m13[:], m2[:], OP.mult)
            nc.vector.tensor_reduce(cacc[f][:], mask[:], mybir.AxisListType.X,
                                    OP.add)
            lw = pix.tile([128, W], f32, tag="lw")
            nc.vector.tensor_tensor(lw[:], E[:], mask[:], OP.mult)
            nc.vector.tensor_reduce(lacc[f][:], lw[:], mybir.AxisListType.X,
                                    OP.add)

        pixel_pre(0)
        pixel_pre(1)

        # ---- main loop: tile t holds pairs q = 4t..4t+3 ----
        # Partition half A = rows 4t..4t+3 (consecutive), half B = rows
        # 128+4t..4t+3+128: 8KB-contiguous DMA runs per partition. x loads
        # alternate between the two HWDGE rings (SP and ACT sequencers).
        # x tiles hold 8 pairs (16KB-contiguous DMA runs per partition --
        # each 8KB descriptor pays ~250ns of unpipelined HBM latency, so
        # longer runs matter); compute stays at 4-pair granularity.
        NT8 = 16
        xts = []
        for t in range(NT8):
            xt = xpool.tile([128, 8, 512], f32, tag="xt", name=f"xt{t}")
            nc.sync.dma_start(out=xt[0:64], in_=seg[:, 8 * t:8 * t + 8, :])
            if t < NT8 - 1:
                nc.sync.dma_start(out=xt[64:128],
                                  in_=seg[:, 128 + 8 * t:128 + 8 * t + 8, :])
            else:
                nc.sync.dma_start(out=xt[64:128, 0:7], in_=seg[:, 248:255, :])
                nc.sync.dma_start(out=xt[64:128, 7:8], in_=seg[:, 254:255, :])
            xts.append(xt)

        def xsrc(q):
            # AP of pair q's row-pair inside its 8-pair tile
            return xts[q // 8][:, q % 8, :]

        for t in range(NT8):
            for half in range(2):
                dt = dpool.tile([128, 4, W], f32, tag="dt")
                for j in range(4):
                    q = 8 * t + 4 * half + j
                    ki = _KIDX[(q // 5, min(q + 128, 254) // 5)]
                    if ki not in cs_tiles:
                        make_cs(ki)
                    cst = get_cs(ki)
                    nc.vector.tensor_tensor(
                        dt[:, j, :].rearrange("p (g f) -> p g f", f=5),
                        xsrc(q)[:, 0:W].rearrange("p (g f) -> p g f", f=5),
                        cst[:, :, None].to_broadcast((128, NG, 5)),
                        OP.subtract,
                    )

                d2t = d2pool.tile([128, 4, W], rdt, tag="d2t")
                nc.scalar.activation(d2t[:], dt[:], AF.Square,
                                     bias=zero_bias[:], scale=1.0)

                for j in range(4):
                    q = 8 * t + 4 * half + j
                    f = q // 64
                    i2 = q % 64
                    nc.tensor.matmul(
                        out=psum_tiles[f][:],
                        lhsT=PB[:, 126 - 2 * i2:254 - 2 * i2],
                        rhs=d2t[:, j, :],
                        start=(i2 == 0), stop=(i2 == 63),
                    )

            if t == 7:
                pixel_finish(0)
            if t == NT8 - 1:
                pixel_finish(1)

        lsum = consts.tile([128, 1], f32, tag="lsum")
        nc.vector.tensor_tensor(lsum[:], lacc[0][:], lacc[1][:], OP.add)
        csum = consts.tile([128, 1], f32, tag="csum")
        nc.vector.tensor_tensor(csum[:], cacc[0][:], cacc[1][:], OP.add)
        nc.sync.dma_start(out=out[:, 0:1], in_=lsum[:])
        nc.sync.dma_start(out=out[:, 1:2], in_=csum[:])

    return nc


def get_module(split_waits=True):
    name = os.environ.get("DGP_REDUCE_DT", "float32r")
    key = (name, split_waits)
    if key not in _NC_CACHE:
        nc = _build_module(name)
        if split_waits:
            _split_excess_waits(nc)
        _NC_CACHE[key] = nc
    return _NC_CACHE[key]


def make_shards(seg_feat, dep_true):
    pb_s = np.zeros((128, 256), np.float32)
    pb_s[0:64, 126] = 1.0
    pb_s[64:128, 127] = 1.0
    rows_fp = np.empty((2, 128), np.int64)
    for f in range(2):
        for p in range(128):
            rows_fp[f, p] = _row_of(f, p)
    crow = np.minimum(5 * (rows_fp // 5) + 2, 254)

    shards = []
    for k in range(8):
        b, h = k // 2, k % 2
        seg_s = np.ascontiguousarray(seg_feat[b, :, 255 * h:255 * h + 255, :],
                                     dtype=np.float32)
        dep_s = np.ascontiguousarray(dep_true[b, 0, 255 * h:255 * h + 255, :],
                                     dtype=np.float32)
        dctr_s = np.ascontiguousarray(dep_s[crow][:, :, 2:512:5][:, :, :NG])
        dpx_s = np.ascontiguousarray(dep_s[rows_fp][:, :, :W])
        dpx_s[1, 127, :] = 0.0  # dup row excluded via the dep>eps mask
        cents = seg_s[:, 2::5, 2:512:5][:, :, :NG]       # [64, 51, 102]
        pcc0_s = np.empty((128, NG), np.float32)
        pcc0_s[0:64] = cents[:, 0]
        pcc0_s[64:128] = cents[:, 25]
        shards.append({
            "seg": seg_s,
            "dpx": dpx_s,
            "dctr": dctr_s,
            "pb": pb_s,
            "pcc0": pcc0_s,
        })
    return shards


def combine(outs):
    tl = 0.0
    tc_ = 0.0
    for o in outs:
        tl += float(np.sum(o["out"][:, 0].astype(np.float64)))
        tc_ += float(np.sum(o["out"][:, 1].astype(np.float64)))
    return np.float32(tl / max(tc_, 1.0))


def kernel(seg_feat, dep_true):
    from concourse.bass_utils import run_bass_kernel_spmd
    nc = get_module()
    shards = make_shards(np.asarray(seg_feat), np.asarray(dep_true))
    res = run_bass_kernel_spmd(nc, shards, list(range(8)))
    return combine(res.results)


# revision 36
# speedup vs baseline: 1.0578x; 1.0578x over previous
"""DGPLoss Trainium2 kernel.

Reference computation (see problem):
  - split [B,C,H,W] into non-overlapping 5x5 patches (510x510 used of 512)
  - seg_sq(p) = sum_c (seg[c, center(p)] - seg[c, p])^2        (C=64)
  - dep_diff(p) = |dep[center(p)] - dep[p]|
  - loss = exp(-dep_diff/10 - seg_sq)
  - mask = (dep_diff > 1e-8) & (sqrt(seg_sq) > 1e-8) & (dep > 1e-8)
    (center pixels fall out automatically: dep_diff == 0 exactly there)
  - out = sum(loss*mask) / max(sum(mask), 1)

Sharding: 8 cores = 4 batches x 2 row-halves (255 rows = 51 strips each).
Each core returns [128, 2] partials (per-psum-partition loss-sum and count);
the host sums and divides.

Per-core layout: pixel-row PAIRS (q, q+128) live on SBUF partitions as
[64ch | 64ch] -- consecutive DRAM rows per partition half, so x-tile DMAs
move 8KB-contiguous runs. DVE subtracts a per-pair patch-center tile
(host-staged, broadcast along the free axis with stride-0 APs), ACT squares
into float32r, and the PE reduces channels with sliding-window ones-block
matmuls (M=128, float32r, full-rate) accumulating 64 matmuls per PSUM fill,
so PSUM partition p of fill f holds pixel row 64f + p//2 + 128*(p%2).
The dep branch uses host-staged center tables; masks/exp/reductions run on
[128, 510] pixel tiles. A post-pass splits semaphore waits onto
EventSemaphore carriers (walrus allows one sync wait per instruction).
"""

import os
import numpy as np
from contextlib import ExitStack

C = 64          # channels
ROWS = 255      # pixel rows per core
W = 510         # used width
NG = 102        # patch groups along w
NTILES = 32     # x tiles (4 row-pairs each)
NPAIRS = 128    # row pairs (q, q+128), incl. dup tail
CHUNK = 17      # center-tile keys per staged chunk

_NC_CACHE = {}


def _row_of(f, p):
    """Global pixel row held by PSUM fill f, partition p (dup row clamped)."""
    r = 64 * f + p // 2 + 128 * (p % 2)
    return min(r, 254)


def _center_keys():
    """Distinct (strip of row q, strip of row q+128) pairs in first-use
    order. Shared by the host shard builder and the kernel builder."""
    keys, index = [], {}
    for q in range(NPAIRS):
        k = (q // 5, min(q + 128, 254) // 5)
        if k not in index:
            index[k] = len(keys)
            keys.append(k)
    return keys, index


_KEYS, _KIDX = _center_keys()
NKEYS = len(_KEYS)          # 51

# walrus codegen in this toolchain allows only ONE sync-wait command on most
# engine instruction structs ("Too many sync wait commands"). Tile's scheduler
# freely emits several. Split the excess onto preceding same-engine
# EventSemaphore instructions (pure wait carriers) -- semantically identical:
# the engine blocks at the same program point either way.
_WAIT_LIMITS = {"ISA": 99}  # raw-encoded payload; do not touch


def _split_excess_waits(nc):
    import json
    import bass_rust

    mj = json.loads(bass_rust.module_to_json_bytes(nc.m))
    n_split = 0
    for fn in mj["functions"]:
        for blk in fn["blocks"]:
            out = []
            for inst in blk["instructions"]:
                si = inst.get("sync_info") or {}
                waits = si.get("on_wait") or []
                limit = _WAIT_LIMITS.get(inst["opcode"], 1)
                if len(waits) > limit:
                    extra, keep = waits[:-limit], waits[-limit:]
                    for i, w in enumerate(extra):
                        out.append({
                            "debug": inst.get("debug"),
                            "engine": inst["engine"],
                            "ins": [], "outs": [],
                            "name": f"{inst['name']}-xw{i}",
                            "opcode": "EventSemaphore",
                            "sync_info": {"on_update": [], "on_wait": [w]},
                        })
                        n_split += 1
                    si["on_wait"] = keep
                out.append(inst)
            blk["instructions"] = out
    nc.m = bass_rust.module_from_json_bytes(json.dumps(mj).encode())
    return n_split


def _build_module(reduce_dt_name="float32r"):
    import concourse.bass as bass
    import concourse.mybir as mybir
    import concourse.tile as tile

    f32 = mybir.dt.float32
    rdt = getattr(mybir.dt, reduce_dt_name)
    AF = mybir.ActivationFunctionType
    OP = mybir.AluOpType

    nc = bass.Bass()
    seg = nc.declare_dram_parameter("seg", (C, ROWS, 512), f32, isOutput=False)
    dpx = nc.declare_dram_parameter("dpx", (2, 128, W), f32, isOutput=False)
    dctr = nc.declare_dram_parameter("dctr", (2, 128, NG), f32, isOutput=False)
    pb = nc.declare_dram_parameter("pb", (128, 256), rdt, isOutput=False)
    pcc0 = nc.declare_dram_parameter("pcc0", (128, NG), f32, isOutput=False)
    out = nc.declare_dram_parameter("out", (128, 2), f32, isOutput=True)

    with tile.TileContext(nc) as tc, ExitStack() as ctx:
        consts = ctx.enter_context(tc.tile_pool(name="consts", bufs=1))
        xpool = ctx.enter_context(tc.tile_pool(name="x", bufs=8))
        dpool = ctx.enter_context(tc.tile_pool(name="d", bufs=3))
        d2pool = ctx.enter_context(tc.tile_pool(name="d2", bufs=3))
        cspool = ctx.enter_context(tc.tile_pool(name="cs", bufs=6))
        pix = ctx.enter_context(tc.tile_pool(name="pix", bufs=2))
        psum = ctx.enter_context(tc.tile_pool(name="psum", bufs=2, space="PSUM"))

        # PB: sliding-window ones-block. Window j = PB[:, 126-2j : 254-2j] is
        # a [128, 128] lhsT whose only nonzeros put (sum over partitions 0:64)
        # in out row 2j and (sum over 64:128) in out row 2j+1; the other 126
        # out rows accumulate zeros. M=128 keeps every matmul's PSUM dst at
        # partition 0 (this walrus rejects col-tiled PSUM offsets). The
        # reduce dtype (float32r) cannot be memset (walrus ISA check), so it
        # is host-staged.
        PB = consts.tile([128, 256], rdt)
        nc.sync.dma_start(out=PB[:], in_=pb[:, :])

        zero_bias = consts.tile([128, 1], f32)
        nc.vector.memset(zero_bias[:], 0.0)
        # comparison thresholds as [128, 1] const tiles: tensor_scalar lowers
        # to an ISA struct with a single sync-wait slot, so the masks use
        # tensor_tensor + broadcast instead.
        eps_d10 = consts.tile([128, 1], f32)
        nc.vector.memset(eps_d10[:], 1e-9)
        eps_ssq = consts.tile([128, 1], f32)
        nc.vector.memset(eps_ssq[:], 1e-16)
        eps_dep = consts.tile([128, 1], f32)
        nc.vector.memset(eps_dep[:], 1e-8)

        # ---- dep pixel tiles, partition p <-> row _row_of(f, p);
        # host-staged pre-permuted (fill-1 partition 127 zeroed on host) ----
        dep_px0 = consts.tile([128, W], f32, tag="dep_px0")
        nc.sync.dma_start(out=dep_px0[:], in_=dpx[0])
        dep_px1 = consts.tile([128, W], f32, tag="dep_px1")
        nc.sync.dma_start(out=dep_px1[:], in_=dpx[1])

        # dep patch-center tiles (host-staged): dpsd[f][p, g] =
        # dep[center_row(_row_of(f, p)), 5g+2]
        dpsd = []
        for f in range(2):
            t = consts.tile([128, NG], f32, tag=f"dpsd{f}", name=f"dpsd{f}")
            nc.sync.dma_start(out=t[:], in_=dctr[f])
            dpsd.append(t)

        # accumulators
        lacc = [consts.tile([128, 1], f32, tag=f"lacc{f}", name=f"lacc{f}")
                for f in range(2)]
        cacc = [consts.tile([128, 1], f32, tag=f"cacc{f}", name=f"cacc{f}")
                for f in range(2)]

        psum_tiles = [psum.tile([128, W], f32, tag="ps", name=f"ps{_f}",
                                padded_shape=[128, 512]) for _f in range(2)]

        # ---- center tiles: extracted on-device from the x tiles by GPSIMD
        # (strided engine reads are fine; a strided DMA gather was not). In
        # this pairing each key's A-half center row (5sa+2 <= 127) sits on
        # partitions 0:64 of its x tile and the B-half row (5sb+2 >= 132) on
        # partitions 64:128 -- both partition-aligned with the cs tile. The
        # one boundary key (0, 25) (B-half center is row 127, an A-half row)
        # is host-staged via pcc0.
        cs_tiles = {}

        def make_cs(ki):
            sa, sb = _KEYS[ki]
            t = cspool.tile([128, NG], f32, tag="cs", name=f"cs_{sa}_{sb}")
            if (sa, sb) == (0, 25):
                nc.sync.dma_start(out=t[:], in_=pcc0[:, :])
            else:
                ra = 5 * sa + 2
                nc.gpsimd.tensor_copy(t[0:64, :],
                                      xts[ra // 4][0:64, ra % 4, 2:W:5])
                qb = 5 * sb + 2 - 128
                nc.gpsimd.tensor_copy(t[64:128, :],
                                      xts[qb // 4][64:128, qb % 4, 2:W:5])
            cs_tiles[ki] = t

        def get_cs(ki):
            return cs_tiles[ki][:, :]

        # pixel phase is split: the dep-only part can run as soon as the dep
        # tiles land (start of kernel); only the psum-dependent part sits on
        # the critical tail after the fill's last matmul.
        dpre = {}

        def pixel_pre(f):
            dep_px = dep_px0 if f == 0 else dep_px1
            ts = pix.tile([128, NG, 5], f32, tag=f"ts{f}", name=f"ts{f}")
            nc.vector.tensor_tensor(
                ts[:],
                dep_px[:].rearrange("p (g f) -> p g f", f=5),
                dpsd[f][:, :, None].to_broadcast((128, NG, 5)),
                OP.subtract,
            )
            d10 = pix.tile([128, W], f32, tag=f"d10{f}", name=f"d10{f}")
            nc.scalar.activation(d10[:], ts[:].rearrange("p g f -> p (g f)"),
                                 AF.Abs, bias=zero_bias[:], scale=0.1)
            m13 = pix.tile([128, W], f32, tag=f"m13{f}", name=f"m13{f}")
            m1 = pix.tile([128, W], f32, tag=f"m1{f}", name=f"m1{f}")
            nc.vector.tensor_tensor(m1[:], d10[:],
                                    eps_d10[:].to_broadcast((128, W)), OP.is_gt)
            m3 = pix.tile([128, W], f32, tag=f"m3{f}", name=f"m3{f}")
            nc.vector.tensor_tensor(m3[:], dep_px[:],
                                    eps_dep[:].to_broadcast((128, W)), OP.is_gt)
            nc.vector.tensor_tensor(m13[:], m1[:], m3[:], OP.mult)
            dpre[f] = (d10, m13)

        def pixel_finish(f):
            d10, m13 = dpre[f]
            u = pix.tile([128, W], f32, tag="u")
            nc.vector.tensor_tensor(u[:], d10[:], psum_tiles[f][:], OP.add)
            E = pix.tile([128, W], f32, tag="E")
            nc.scalar.activation(E[:], u[:], AF.Exp, bias=zero_bias[:],
                                 scale=-1.0)
            m2 = pix.tile([128, W], f32, tag="m2")
            nc.vector.tensor_tensor(m2[:], psum_tiles[f][:],
                                    eps_ssq[:].to_broadcast((128, W)), OP.is_gt)
            mask = pix.tile([128, W], f32, tag="mask")
            nc.vector.tensor_tensor(mask[:], m13[:], m2[:], OP.mult)
            nc.vector.tensor_reduce(cacc[f][:], mask[:], mybir.AxisListType.X,
                                    OP.add)
            lw = pix.tile([128, W], f32, tag="lw")
            nc.vector.tensor_tensor(lw[:], E[:], mask[:], OP.mult)
            nc.vector.tensor_reduce(lacc[f][:], lw[:], mybir.AxisListType.X,
                                    OP.add)

        pixel_pre(0)
        pixel_pre(1)

        # ---- main loop: tile t holds pairs q = 4t..4t+3 ----
        # Partition half A = rows 4t..4t+3 (consecutive), half B = rows
        # 128+4t..4t+3+128: 8KB-contiguous DMA runs per partition. x loads
        # alternate between the two HWDGE rings (SP and ACT sequencers).
        xts = []
        for t in range(NTILES):
            dma = nc.sync.dma_start
            xt = xpool.tile([128, 4, 512], f32, tag="xt", name=f"xt{t}")
            dma(out=xt[0:64], in_=seg[:, 4 * t:4 * t + 4, :])
            if t < NTILES - 1:
                dma(out=xt[64:128],
                    in_=seg[:, 128 + 4 * t:128 + 4 * t + 4, :])
            else:
                dma(out=xt[64:128, 0:3], in_=seg[:, 252:255, :])
                dma(out=xt[64:128, 3:4], in_=seg[:, 254:255, :])
            xts.append(xt)

        for t in range(NTILES):
            xt = xts[t]
            for j in range(4):
                q = 4 * t + j
                ki = _KIDX[(q // 5, min(q + 128, 254) // 5)]
                if ki not in cs_tiles:
                    make_cs(ki)

            dt = dpool.tile([128, 4, W], f32, tag="dt")
            for j in range(4):
                q = 4 * t + j
                cst = get_cs(_KIDX[(q // 5, min(q + 128, 254) // 5)])
                nc.vector.tensor_tensor(
                    dt[:, j, :].rearrange("p (g f) -> p g f", f=5),
                    xt[:, j, 0:W].rearrange("p (g f) -> p g f", f=5),
                    cst[:, :, None].to_broadcast((128, NG, 5)),
                    OP.subtract,
                )

            d2t = d2pool.tile([128, 4, W], rdt, tag="d2t")
            nc.scalar.activation(d2t[:], dt[:], AF.Square,
                                 bias=zero_bias[:], scale=1.0)

            for j in range(4):
                q = 4 * t + j
                f = q // 64
                i2 = q % 64
                nc.tensor.matmul(
                    out=psum_tiles[f][:],
                    lhsT=PB[:, 126 - 2 * i2:254 - 2 * i2],
                    rhs=d2t[:, j, :],
                    start=(i2 == 0), stop=(i2 == 63),
                )

            if t == 15:
                pixel_finish(0)
            if t == NTILES - 1:
                pixel_finish(1)

        lsum = consts.tile([128, 1], f32, tag="lsum")
        nc.vector.tensor_tensor(lsum[:], lacc[0][:], lacc[1][:], OP.add)
        csum = consts.tile([128, 1], f32, tag="csum")
        nc.vector.tensor_tensor(csum[:], cacc[0][:], cacc[1][:], OP.add)
        nc.sync.dma_start(out=out[:, 0:1], in_=lsum[:])
        nc.sync.dma_start(out=out[:, 1:2], in_=csum[:])

    return nc


def get_module(split_waits=True):
    name = os.environ.get("DGP_REDUCE_DT", "float32r")
    key = (name, split_waits)
    if key not in _NC_CACHE:
        nc = _build_module(name)
        if split_waits:
            _split_excess_waits(nc)
        _NC_CACHE[key] = nc
    return _NC_CACHE[key]


def make_shards(seg_feat, dep_true):
    pb_s = np.zeros((128, 256), np.float32)
    pb_s[0:64, 126] = 1.0
    pb_s[64:128, 127] = 1.0
    rows_fp = np.empty((2, 128), np.int64)
    for f in range(2):
        for p in range(128):
            rows_fp[f, p] = _row_of(f, p)
    crow = np.minimum(5 * (rows_fp // 5) + 2, 254)

    shards = []
    for k in range(8):
        b, h = k // 2, k % 2
        seg_s = np.ascontiguousarray(seg_feat[b, :, 255 * h:255 * h + 255, :],
                                     dtype=np.float32)
        dep_s = np.ascontiguousarray(dep_true[b, 0, 255 * h:255 * h + 255, :],
                                     dtype=np.float32)
        dctr_s = np.ascontiguousarray(dep_s[crow][:, :, 2:512:5][:, :, :NG])
        dpx_s = np.ascontiguousarray(dep_s[rows_fp][:, :, :W])
        dpx_s[1, 127, :] = 0.0  # dup row excluded via the dep>eps mask
        cents = seg_s[:, 2::5, 2:512:5][:, :, :NG]       # [64, 51, 102]
        pcc0_s = np.empty((128, NG), np.float32)
        pcc0_s[0:64] = cents[:, 0]
        pcc0_s[64:128] = cents[:, 25]
        shards.append({
            "seg": seg_s,
            "dpx": dpx_s,
            "dctr": dctr_s,
            "pb": pb_s,
            "pcc0": pcc0_s,
        })
    return shards


def combine(outs):
    tl = 0.0
    tc_ = 0.0
    for o in outs:
        tl += float(np.sum(o["out"][:, 0].astype(np.float64)))
        tc_ += float(np.sum(o["out"][:, 1].astype(np.float64)))
    return np.float32(tl / max(tc_, 1.0))


def kernel(seg_feat, dep_true):
    from concourse.bass_utils import run_bass_kernel_spmd
    nc = get_module()
    shards = make_shards(np.asarray(seg_feat), np.asarray(dep_true))
    res = run_bass_kernel_spmd(nc, shards, list(range(8)))
    return combine(res.results)


# revision 37
# speedup vs baseline: 1.3214x; 1.2493x over previous
"""DGPLoss Trainium2 kernel.

Reference computation (see problem):
  - split [B,C,H,W] into non-overlapping 5x5 patches (510x510 used of 512)
  - seg_sq(p) = sum_c (seg[c, center(p)] - seg[c, p])^2        (C=64)
  - dep_diff(p) = |dep[center(p)] - dep[p]|
  - loss = exp(-dep_diff/10 - seg_sq)
  - mask = (dep_diff > 1e-8) & (sqrt(seg_sq) > 1e-8) & (dep > 1e-8)
    (center pixels fall out automatically: dep_diff == 0 exactly there)
  - out = sum(loss*mask) / max(sum(mask), 1)

Sharding: 8 cores = 4 batches x 2 row-halves (255 rows = 51 strips each).
Each core returns [128, 2] partials (per-psum-partition loss-sum and count);
the host sums and divides.

Per-core layout: pixel-row PAIRS (q, q+128) live on SBUF partitions as
[64ch | 64ch] -- consecutive DRAM rows per partition half, so x-tile DMAs
move 8KB-contiguous runs. DVE subtracts a per-pair patch-center tile
(host-staged, broadcast along the free axis with stride-0 APs), ACT squares
into float32r, and the PE reduces channels with sliding-window ones-block
matmuls (M=128, float32r, full-rate) accumulating 64 matmuls per PSUM fill,
so PSUM partition p of fill f holds pixel row 64f + p//2 + 128*(p%2).
The dep branch uses host-staged center tables; masks/exp/reductions run on
[128, 510] pixel tiles. A post-pass splits semaphore waits onto
EventSemaphore carriers (walrus allows one sync wait per instruction).
"""

import os
import numpy as np
from contextlib import ExitStack

C = 64          # channels
ROWS = 255      # pixel rows per core
W = 510         # used width
NG = 102        # patch groups along w
NTILES = 32     # x tiles (4 row-pairs each)
NPAIRS = 128    # row pairs (q, q+128), incl. dup tail
CHUNK = 17      # center-tile keys per staged chunk

_NC_CACHE = {}


def _row_of(f, p):
    """Global pixel row held by PSUM fill f, partition p (dup row clamped)."""
    r = 64 * f + p // 2 + 128 * (p % 2)
    return min(r, 254)


def _center_keys():
    """Distinct (strip of row q, strip of row q+128) pairs in first-use
    order. Shared by the host shard builder and the kernel builder."""
    keys, index = [], {}
    for q in range(NPAIRS):
        k = (q // 5, min(q + 128, 254) // 5)
        if k not in index:
            index[k] = len(keys)
            keys.append(k)
    return keys, index


_KEYS, _KIDX = _center_keys()
NKEYS = len(_KEYS)          # 51

# walrus codegen in this toolchain allows only ONE sync-wait command on most
# engine instruction structs ("Too many sync wait commands"). Tile's scheduler
# freely emits several. Split the excess onto preceding same-engine
# EventSemaphore instructions (pure wait carriers) -- semantically identical:
# the engine blocks at the same program point either way.
_WAIT_LIMITS = {"ISA": 99}  # raw-encoded payload; do not touch


def _split_excess_waits(nc):
    import json
    import bass_rust

    mj = json.loads(bass_rust.module_to_json_bytes(nc.m))
    n_split = 0
    for fn in mj["functions"]:
        for blk in fn["blocks"]:
            out = []
            for inst in blk["instructions"]:
                si = inst.get("sync_info") or {}
                waits = si.get("on_wait") or []
                limit = _WAIT_LIMITS.get(inst["opcode"], 1)
                if len(waits) > limit:
                    extra, keep = waits[:-limit], waits[-limit:]
                    for i, w in enumerate(extra):
                        out.append({
                            "debug": inst.get("debug"),
                            "engine": inst["engine"],
                            "ins": [], "outs": [],
                            "name": f"{inst['name']}-xw{i}",
                            "opcode": "EventSemaphore",
                            "sync_info": {"on_update": [], "on_wait": [w]},
                        })
                        n_split += 1
                    si["on_wait"] = keep
                out.append(inst)
            blk["instructions"] = out
    nc.m = bass_rust.module_from_json_bytes(json.dumps(mj).encode())
    return n_split


def _build_module(reduce_dt_name="float32r"):
    import concourse.bass as bass
    import concourse.mybir as mybir
    import concourse.tile as tile

    f32 = mybir.dt.float32
    rdt = getattr(mybir.dt, reduce_dt_name)
    AF = mybir.ActivationFunctionType
    OP = mybir.AluOpType

    nc = bass.Bass()
    xin = nc.declare_dram_parameter("xin", (NTILES, 128, 4, 512), f32,
                                    isOutput=False)
    dpx = nc.declare_dram_parameter("dpx", (2, 128, W), f32, isOutput=False)
    dctr = nc.declare_dram_parameter("dctr", (2, 128, NG), f32, isOutput=False)
    pb = nc.declare_dram_parameter("pb", (128, 256), rdt, isOutput=False)
    pcc0 = nc.declare_dram_parameter("pcc0", (128, NG), f32, isOutput=False)
    out = nc.declare_dram_parameter("out", (128, 2), f32, isOutput=True)

    with tile.TileContext(nc) as tc, ExitStack() as ctx:
        consts = ctx.enter_context(tc.tile_pool(name="consts", bufs=1))
        xpool = ctx.enter_context(tc.tile_pool(name="x", bufs=8))
        dpool = ctx.enter_context(tc.tile_pool(name="d", bufs=3))
        d2pool = ctx.enter_context(tc.tile_pool(name="d2", bufs=3))
        cspool = ctx.enter_context(tc.tile_pool(name="cs", bufs=6))
        pix = ctx.enter_context(tc.tile_pool(name="pix", bufs=2))
        psum = ctx.enter_context(tc.tile_pool(name="psum", bufs=2, space="PSUM"))

        # PB: sliding-window ones-block. Window j = PB[:, 126-2j : 254-2j] is
        # a [128, 128] lhsT whose only nonzeros put (sum over partitions 0:64)
        # in out row 2j and (sum over 64:128) in out row 2j+1; the other 126
        # out rows accumulate zeros. M=128 keeps every matmul's PSUM dst at
        # partition 0 (this walrus rejects col-tiled PSUM offsets). The
        # reduce dtype (float32r) cannot be memset (walrus ISA check), so it
        # is host-staged.
        PB = consts.tile([128, 256], rdt)
        nc.sync.dma_start(out=PB[:], in_=pb[:, :])

        zero_bias = consts.tile([128, 1], f32)
        nc.vector.memset(zero_bias[:], 0.0)
        # comparison thresholds as [128, 1] const tiles: tensor_scalar lowers
        # to an ISA struct with a single sync-wait slot, so the masks use
        # tensor_tensor + broadcast instead.
        eps_d10 = consts.tile([128, 1], f32)
        nc.vector.memset(eps_d10[:], 1e-9)
        eps_ssq = consts.tile([128, 1], f32)
        nc.vector.memset(eps_ssq[:], 1e-16)
        eps_dep = consts.tile([128, 1], f32)
        nc.vector.memset(eps_dep[:], 1e-8)

        # ---- dep pixel tiles, partition p <-> row _row_of(f, p);
        # host-staged pre-permuted (fill-1 partition 127 zeroed on host) ----
        dep_px0 = consts.tile([128, W], f32, tag="dep_px0")
        nc.sync.dma_start(out=dep_px0[:], in_=dpx[0])
        dep_px1 = consts.tile([128, W], f32, tag="dep_px1")
        nc.sync.dma_start(out=dep_px1[:], in_=dpx[1])

        # dep patch-center tiles (host-staged): dpsd[f][p, g] =
        # dep[center_row(_row_of(f, p)), 5g+2]
        dpsd = []
        for f in range(2):
            t = consts.tile([128, NG], f32, tag=f"dpsd{f}", name=f"dpsd{f}")
            nc.sync.dma_start(out=t[:], in_=dctr[f])
            dpsd.append(t)

        # accumulators
        lacc = [consts.tile([128, 1], f32, tag=f"lacc{f}", name=f"lacc{f}")
                for f in range(2)]
        cacc = [consts.tile([128, 1], f32, tag=f"cacc{f}", name=f"cacc{f}")
                for f in range(2)]

        psum_tiles = [psum.tile([128, W], f32, tag="ps", name=f"ps{_f}",
                                padded_shape=[128, 512]) for _f in range(2)]

        # ---- center tiles: extracted on-device from the x tiles by GPSIMD
        # (strided engine reads are fine; a strided DMA gather was not). In
        # this pairing each key's A-half center row (5sa+2 <= 127) sits on
        # partitions 0:64 of its x tile and the B-half row (5sb+2 >= 132) on
        # partitions 64:128 -- both partition-aligned with the cs tile. The
        # one boundary key (0, 25) (B-half center is row 127, an A-half row)
        # is host-staged via pcc0.
        cs_tiles = {}

        def make_cs(ki):
            sa, sb = _KEYS[ki]
            t = cspool.tile([128, NG], f32, tag="cs", name=f"cs_{sa}_{sb}")
            if (sa, sb) == (0, 25):
                nc.sync.dma_start(out=t[:], in_=pcc0[:, :])
            else:
                ra = 5 * sa + 2
                nc.gpsimd.tensor_copy(t[0:64, :],
                                      xts[ra // 4][0:64, ra % 4, 2:W:5])
                qb = 5 * sb + 2 - 128
                nc.gpsimd.tensor_copy(t[64:128, :],
                                      xts[qb // 4][64:128, qb % 4, 2:W:5])
            cs_tiles[ki] = t

        def get_cs(ki):
            return cs_tiles[ki][:, :]

        # pixel phase is split: the dep-only part can run as soon as the dep
        # tiles land (start of kernel); only the psum-dependent part sits on
        # the critical tail after the fill's last matmul.
        dpre = {}

        def pixel_pre(f):
            dep_px = dep_px0 if f == 0 else dep_px1
            ts = pix.tile([128, NG, 5], f32, tag=f"ts{f}", name=f"ts{f}")
            nc.vector.tensor_tensor(
                ts[:],
                dep_px[:].rearrange("p (g f) -> p g f", f=5),
                dpsd[f][:, :, None].to_broadcast((128, NG, 5)),
                OP.subtract,
            )
            d10 = pix.tile([128, W], f32, tag=f"d10{f}", name=f"d10{f}")
            nc.scalar.activation(d10[:], ts[:].rearrange("p g f -> p (g f)"),
                                 AF.Abs, bias=zero_bias[:], scale=0.1)
            m13 = pix.tile([128, W], f32, tag=f"m13{f}", name=f"m13{f}")
            m1 = pix.tile([128, W], f32, tag=f"m1{f}", name=f"m1{f}")
            nc.vector.tensor_tensor(m1[:], d10[:],
                                    eps_d10[:].to_broadcast((128, W)), OP.is_gt)
            m3 = pix.tile([128, W], f32, tag=f"m3{f}", name=f"m3{f}")
            nc.vector.tensor_tensor(m3[:], dep_px[:],
                                    eps_dep[:].to_broadcast((128, W)), OP.is_gt)
            nc.vector.tensor_tensor(m13[:], m1[:], m3[:], OP.mult)
            dpre[f] = (d10, m13)

        def pixel_finish(f):
            d10, m13 = dpre[f]
            u = pix.tile([128, W], f32, tag="u")
            nc.vector.tensor_tensor(u[:], d10[:], psum_tiles[f][:], OP.add)
            E = pix.tile([128, W], f32, tag="E")
            nc.scalar.activation(E[:], u[:], AF.Exp, bias=zero_bias[:],
                                 scale=-1.0)
            m2 = pix.tile([128, W], f32, tag="m2")
            nc.vector.tensor_tensor(m2[:], psum_tiles[f][:],
                                    eps_ssq[:].to_broadcast((128, W)), OP.is_gt)
            mask = pix.tile([128, W], f32, tag="mask")
            nc.vector.tensor_tensor(mask[:], m13[:], m2[:], OP.mult)
            nc.vector.tensor_reduce(cacc[f][:], mask[:], mybir.AxisListType.X,
                                    OP.add)
            lw = pix.tile([128, W], f32, tag="lw")
            nc.vector.tensor_tensor(lw[:], E[:], mask[:], OP.mult)
            nc.vector.tensor_reduce(lacc[f][:], lw[:], mybir.AxisListType.X,
                                    OP.add)

        pixel_pre(0)
        pixel_pre(1)

        # ---- main loop: tile t holds pairs q = 4t..4t+3 ----
        # Partition half A = rows 4t..4t+3 (consecutive), half B = rows
        # 128+4t..4t+3+128: 8KB-contiguous DMA runs per partition. x loads
        # alternate between the two HWDGE rings (SP and ACT sequencers).
        # x input is host-permuted tile-major: each tile's 1MB is contiguous
        # in DRAM (HBM row locality; one DMA instruction per tile).
        xts = []
        for t in range(NTILES):
            xt = xpool.tile([128, 4, 512], f32, tag="xt", name=f"xt{t}")
            nc.sync.dma_start(out=xt[:], in_=xin[t])
            xts.append(xt)

        for t in range(NTILES):
            xt = xts[t]
            for j in range(4):
                q = 4 * t + j
                ki = _KIDX[(q // 5, min(q + 128, 254) // 5)]
                if ki not in cs_tiles:
                    make_cs(ki)

            dt = dpool.tile([128, 4, W], f32, tag="dt")
            for j in range(4):
                q = 4 * t + j
                cst = get_cs(_KIDX[(q // 5, min(q + 128, 254) // 5)])
                nc.vector.tensor_tensor(
                    dt[:, j, :].rearrange("p (g f) -> p g f", f=5),
                    xt[:, j, 0:W].rearrange("p (g f) -> p g f", f=5),
                    cst[:, :, None].to_broadcast((128, NG, 5)),
                    OP.subtract,
                )

            d2t = d2pool.tile([128, 4, W], rdt, tag="d2t")
            nc.scalar.activation(d2t[:], dt[:], AF.Square,
                                 bias=zero_bias[:], scale=1.0)

            for j in range(4):
                q = 4 * t + j
                f = q // 64
                i2 = q % 64
                nc.tensor.matmul(
                    out=psum_tiles[f][:],
                    lhsT=PB[:, 126 - 2 * i2:254 - 2 * i2],
                    rhs=d2t[:, j, :],
                    start=(i2 == 0), stop=(i2 == 63),
                )

            if t == 15:
                pixel_finish(0)
            if t == NTILES - 1:
                pixel_finish(1)

        lsum = consts.tile([128, 1], f32, tag="lsum")
        nc.vector.tensor_tensor(lsum[:], lacc[0][:], lacc[1][:], OP.add)
        csum = consts.tile([128, 1], f32, tag="csum")
        nc.vector.tensor_tensor(csum[:], cacc[0][:], cacc[1][:], OP.add)
        nc.sync.dma_start(out=out[:, 0:1], in_=lsum[:])
        nc.sync.dma_start(out=out[:, 1:2], in_=csum[:])

    return nc


def get_module(split_waits=True):
    name = os.environ.get("DGP_REDUCE_DT", "float32r")
    key = (name, split_waits)
    if key not in _NC_CACHE:
        nc = _build_module(name)
        if split_waits:
            _split_excess_waits(nc)
        _NC_CACHE[key] = nc
    return _NC_CACHE[key]


def make_shards(seg_feat, dep_true):
    pb_s = np.zeros((128, 256), np.float32)
    pb_s[0:64, 126] = 1.0
    pb_s[64:128, 127] = 1.0
    rows_fp = np.empty((2, 128), np.int64)
    for f in range(2):
        for p in range(128):
            rows_fp[f, p] = _row_of(f, p)
    crow = np.minimum(5 * (rows_fp // 5) + 2, 254)

    shards = []
    for k in range(8):
        b, h = k // 2, k % 2
        seg_s = np.asarray(seg_feat[b, :, 255 * h:255 * h + 255, :],
                           dtype=np.float32)
        rowsA = np.arange(128).reshape(32, 4)
        rowsB = np.minimum(128 + np.arange(128), 254).reshape(32, 4)
        xa = seg_s[:, rowsA, :].transpose(1, 0, 2, 3)    # [32, 64, 4, 512]
        xb = seg_s[:, rowsB, :].transpose(1, 0, 2, 3)
        xin_s = np.ascontiguousarray(
            np.concatenate([xa[:, None], xb[:, None]], axis=1)
            .reshape(NTILES, 128, 4, 512))
        dep_s = np.ascontiguousarray(dep_true[b, 0, 255 * h:255 * h + 255, :],
                                     dtype=np.float32)
        dctr_s = np.ascontiguousarray(dep_s[crow][:, :, 2:512:5][:, :, :NG])
        dpx_s = np.ascontiguousarray(dep_s[rows_fp][:, :, :W])
        dpx_s[1, 127, :] = 0.0  # dup row excluded via the dep>eps mask
        cents = seg_s[:, 2::5, 2:512:5][:, :, :NG]       # [64, 51, 102]
        pcc0_s = np.empty((128, NG), np.float32)
        pcc0_s[0:64] = cents[:, 0]
        pcc0_s[64:128] = cents[:, 25]
        shards.append({
            "xin": xin_s,
            "dpx": dpx_s,
            "dctr": dctr_s,
            "pb": pb_s,
            "pcc0": pcc0_s,
        })
    return shards


def combine(outs):
    tl = 0.0
    tc_ = 0.0
    for o in outs:
        tl += float(np.sum(o["out"][:, 0].astype(np.float64)))
        tc_ += float(np.sum(o["out"][:, 1].astype(np.float64)))
    return np.float32(tl / max(tc_, 1.0))


def kernel(seg_feat, dep_true):
    from concourse.bass_utils import run_bass_kernel_spmd
    nc = get_module()
    shards = make_shards(np.asarray(seg_feat), np.asarray(dep_true))
    res = run_bass_kernel_spmd(nc, shards, list(range(8)))
    return combine(res.results)


# revision 38
# speedup vs baseline: 1.3486x; 1.0206x over previous
"""DGPLoss Trainium2 kernel.

Reference computation (see problem):
  - split [B,C,H,W] into non-overlapping 5x5 patches (510x510 used of 512)
  - seg_sq(p) = sum_c (seg[c, center(p)] - seg[c, p])^2        (C=64)
  - dep_diff(p) = |dep[center(p)] - dep[p]|
  - loss = exp(-dep_diff/10 - seg_sq)
  - mask = (dep_diff > 1e-8) & (sqrt(seg_sq) > 1e-8) & (dep > 1e-8)
    (center pixels fall out automatically: dep_diff == 0 exactly there)
  - out = sum(loss*mask) / max(sum(mask), 1)

Sharding: 8 cores = 4 batches x 2 row-halves (255 rows = 51 strips each).
Each core returns [128, 2] partials (per-psum-partition loss-sum and count);
the host sums and divides.

Per-core layout: pixel-row PAIRS (q, q+128) live on SBUF partitions as
[64ch | 64ch] -- consecutive DRAM rows per partition half, so x-tile DMAs
move 8KB-contiguous runs. DVE subtracts a per-pair patch-center tile
(host-staged, broadcast along the free axis with stride-0 APs), ACT squares
into float32r, and the PE reduces channels with sliding-window ones-block
matmuls (M=128, float32r, full-rate) accumulating 64 matmuls per PSUM fill,
so PSUM partition p of fill f holds pixel row 64f + p//2 + 128*(p%2).
The dep branch uses host-staged center tables; masks/exp/reductions run on
[128, 510] pixel tiles. A post-pass splits semaphore waits onto
EventSemaphore carriers (walrus allows one sync wait per instruction).
"""

import os
import numpy as np
from contextlib import ExitStack

C = 64          # channels
ROWS = 255      # pixel rows per core
W = 510         # used width
NG = 102        # patch groups along w
NTILES = 32     # x tiles (4 row-pairs each)
NPAIRS = 128    # row pairs (q, q+128), incl. dup tail
CHUNK = 17      # center-tile keys per staged chunk

_NC_CACHE = {}


def _row_of(f, p):
    """Global pixel row held by PSUM fill f, partition p (dup row clamped)."""
    r = 64 * f + p // 2 + 128 * (p % 2)
    return min(r, 254)


def _center_keys():
    """Distinct (strip of row q, strip of row q+128) pairs in first-use
    order. Shared by the host shard builder and the kernel builder."""
    keys, index = [], {}
    for q in range(NPAIRS):
        k = (q // 5, min(q + 128, 254) // 5)
        if k not in index:
            index[k] = len(keys)
            keys.append(k)
    return keys, index


_KEYS, _KIDX = _center_keys()
NKEYS = len(_KEYS)          # 51

# walrus codegen in this toolchain allows only ONE sync-wait command on most
# engine instruction structs ("Too many sync wait commands"). Tile's scheduler
# freely emits several. Split the excess onto preceding same-engine
# EventSemaphore instructions (pure wait carriers) -- semantically identical:
# the engine blocks at the same program point either way.
_WAIT_LIMITS = {"ISA": 99}  # raw-encoded payload; do not touch


def _split_excess_waits(nc):
    import json
    import bass_rust

    mj = json.loads(bass_rust.module_to_json_bytes(nc.m))
    n_split = 0
    for fn in mj["functions"]:
        for blk in fn["blocks"]:
            out = []
            for inst in blk["instructions"]:
                si = inst.get("sync_info") or {}
                waits = si.get("on_wait") or []
                limit = _WAIT_LIMITS.get(inst["opcode"], 1)
                if len(waits) > limit:
                    extra, keep = waits[:-limit], waits[-limit:]
                    for i, w in enumerate(extra):
                        out.append({
                            "debug": inst.get("debug"),
                            "engine": inst["engine"],
                            "ins": [], "outs": [],
                            "name": f"{inst['name']}-xw{i}",
                            "opcode": "EventSemaphore",
                            "sync_info": {"on_update": [], "on_wait": [w]},
                        })
                        n_split += 1
                    si["on_wait"] = keep
                out.append(inst)
            blk["instructions"] = out
    nc.m = bass_rust.module_from_json_bytes(json.dumps(mj).encode())
    return n_split


def _build_module(reduce_dt_name="float32r"):
    import concourse.bass as bass
    import concourse.mybir as mybir
    import concourse.tile as tile

    f32 = mybir.dt.float32
    rdt = getattr(mybir.dt, reduce_dt_name)
    AF = mybir.ActivationFunctionType
    OP = mybir.AluOpType

    nc = bass.Bass()
    xin = nc.declare_dram_parameter("xin", (NTILES, 128, 4, 512), f32,
                                    isOutput=False)
    dpx = nc.declare_dram_parameter("dpx", (2, 128, W), f32, isOutput=False)
    dctr = nc.declare_dram_parameter("dctr", (2, 128, NG), f32, isOutput=False)
    pb = nc.declare_dram_parameter("pb", (128, 256), rdt, isOutput=False)
    pcc0 = nc.declare_dram_parameter("pcc0", (128, NG), f32, isOutput=False)
    out = nc.declare_dram_parameter("out", (128, 2), f32, isOutput=True)

    with tile.TileContext(nc) as tc, ExitStack() as ctx:
        consts = ctx.enter_context(tc.tile_pool(name="consts", bufs=1))
        xpool = ctx.enter_context(tc.tile_pool(name="x", bufs=8))
        dpool = ctx.enter_context(tc.tile_pool(name="d", bufs=3))
        d2pool = ctx.enter_context(tc.tile_pool(name="d2", bufs=3))
        cspool = ctx.enter_context(tc.tile_pool(name="cs", bufs=6))
        pix = ctx.enter_context(tc.tile_pool(name="pix", bufs=2))
        psum = ctx.enter_context(tc.tile_pool(name="psum", bufs=2, space="PSUM"))

        # PB: sliding-window ones-block. Window j = PB[:, 126-2j : 254-2j] is
        # a [128, 128] lhsT whose only nonzeros put (sum over partitions 0:64)
        # in out row 2j and (sum over 64:128) in out row 2j+1; the other 126
        # out rows accumulate zeros. M=128 keeps every matmul's PSUM dst at
        # partition 0 (this walrus rejects col-tiled PSUM offsets). The
        # reduce dtype (float32r) cannot be memset (walrus ISA check), so it
        # is host-staged.
        PB = consts.tile([128, 256], rdt)
        nc.sync.dma_start(out=PB[:], in_=pb[:, :])

        zero_bias = consts.tile([128, 1], f32)
        nc.vector.memset(zero_bias[:], 0.0)
        # comparison thresholds as [128, 1] const tiles: tensor_scalar lowers
        # to an ISA struct with a single sync-wait slot, so the masks use
        # tensor_tensor + broadcast instead.
        eps_d10 = consts.tile([128, 1], f32)
        nc.vector.memset(eps_d10[:], 1e-9)
        eps_ssq = consts.tile([128, 1], f32)
        nc.vector.memset(eps_ssq[:], 1e-16)
        eps_dep = consts.tile([128, 1], f32)
        nc.vector.memset(eps_dep[:], 1e-8)

        # ---- dep pixel tiles, partition p <-> row _row_of(f, p);
        # host-staged pre-permuted (fill-1 partition 127 zeroed on host) ----
        dep_px0 = consts.tile([128, W], f32, tag="dep_px0")
        nc.sync.dma_start(out=dep_px0[:], in_=dpx[0])
        dep_px1 = consts.tile([128, W], f32, tag="dep_px1")
        nc.sync.dma_start(out=dep_px1[:], in_=dpx[1])

        # dep patch-center tiles (host-staged): dpsd[f][p, g] =
        # dep[center_row(_row_of(f, p)), 5g+2]
        dpsd = []
        for f in range(2):
            t = consts.tile([128, NG], f32, tag=f"dpsd{f}", name=f"dpsd{f}")
            nc.sync.dma_start(out=t[:], in_=dctr[f])
            dpsd.append(t)

        # accumulators
        lacc = [consts.tile([128, 1], f32, tag=f"lacc{f}", name=f"lacc{f}")
                for f in range(2)]
        cacc = [consts.tile([128, 1], f32, tag=f"cacc{f}", name=f"cacc{f}")
                for f in range(2)]

        psum_tiles = [psum.tile([128, W], f32, tag="ps", name=f"ps{_f}",
                                padded_shape=[128, 512]) for _f in range(2)]

        # ---- center tiles: extracted on-device from the x tiles by GPSIMD
        # (strided engine reads are fine; a strided DMA gather was not). In
        # this pairing each key's A-half center row (5sa+2 <= 127) sits on
        # partitions 0:64 of its x tile and the B-half row (5sb+2 >= 132) on
        # partitions 64:128 -- both partition-aligned with the cs tile. The
        # one boundary key (0, 25) (B-half center is row 127, an A-half row)
        # is host-staged via pcc0.
        cs_tiles = {}

        def make_cs(ki):
            # copies split ACT/GPSIMD to balance engine load (DVE is the
            # bottleneck; ACT copy 272ns vs gpsimd 754ns per [64, 102])
            sa, sb = _KEYS[ki]
            t = cspool.tile([128, NG], f32, tag="cs", name=f"cs_{sa}_{sb}")
            if (sa, sb) == (0, 25):
                nc.sync.dma_start(out=t[:], in_=pcc0[:, :])
            else:
                cp = nc.scalar.copy if ki % 4 != 3 else nc.gpsimd.tensor_copy
                ra = 5 * sa + 2
                cp(t[0:64, :], xts[ra // 4][0:64, ra % 4, 2:W:5])
                qb = 5 * sb + 2 - 128
                cp(t[64:128, :], xts[qb // 4][64:128, qb % 4, 2:W:5])
            cs_tiles[ki] = t

        def get_cs(ki):
            return cs_tiles[ki][:, :]

        # pixel phase is split: the dep-only part can run as soon as the dep
        # tiles land (start of kernel); only the psum-dependent part sits on
        # the critical tail after the fill's last matmul.
        dpre = {}

        def pixel_pre(f):
            dep_px = dep_px0 if f == 0 else dep_px1
            ts = pix.tile([128, NG, 5], f32, tag=f"ts{f}", name=f"ts{f}")
            nc.vector.tensor_tensor(
                ts[:],
                dep_px[:].rearrange("p (g f) -> p g f", f=5),
                dpsd[f][:, :, None].to_broadcast((128, NG, 5)),
                OP.subtract,
            )
            d10 = pix.tile([128, W], f32, tag=f"d10{f}", name=f"d10{f}")
            nc.scalar.activation(d10[:], ts[:].rearrange("p g f -> p (g f)"),
                                 AF.Abs, bias=zero_bias[:], scale=0.1)
            m13 = pix.tile([128, W], f32, tag=f"m13{f}", name=f"m13{f}")
            m1 = pix.tile([128, W], f32, tag=f"m1{f}", name=f"m1{f}")
            nc.vector.tensor_tensor(m1[:], d10[:],
                                    eps_d10[:].to_broadcast((128, W)), OP.is_gt)
            m3 = pix.tile([128, W], f32, tag=f"m3{f}", name=f"m3{f}")
            nc.vector.tensor_tensor(m3[:], dep_px[:],
                                    eps_dep[:].to_broadcast((128, W)), OP.is_gt)
            nc.vector.tensor_tensor(m13[:], m1[:], m3[:], OP.mult)
            dpre[f] = (d10, m13)

        def pixel_finish(f):
            d10, m13 = dpre[f]
            u = pix.tile([128, W], f32, tag="u")
            nc.vector.tensor_tensor(u[:], d10[:], psum_tiles[f][:], OP.add)
            E = pix.tile([128, W], f32, tag="E")
            nc.scalar.activation(E[:], u[:], AF.Exp, bias=zero_bias[:],
                                 scale=-1.0)
            m2 = pix.tile([128, W], f32, tag="m2")
            nc.vector.tensor_tensor(m2[:], psum_tiles[f][:],
                                    eps_ssq[:].to_broadcast((128, W)), OP.is_gt)
            mask = pix.tile([128, W], f32, tag="mask")
            nc.vector.tensor_tensor(mask[:], m13[:], m2[:], OP.mult)
            nc.vector.tensor_reduce(cacc[f][:], mask[:], mybir.AxisListType.X,
                                    OP.add)
            lw = pix.tile([128, W], f32, tag="lw")
            nc.vector.tensor_tensor(lw[:], E[:], mask[:], OP.mult)
            nc.vector.tensor_reduce(lacc[f][:], lw[:], mybir.AxisListType.X,
                                    OP.add)

        pixel_pre(0)
        pixel_pre(1)

        # ---- main loop: tile t holds pairs q = 4t..4t+3 ----
        # Partition half A = rows 4t..4t+3 (consecutive), half B = rows
        # 128+4t..4t+3+128: 8KB-contiguous DMA runs per partition. x loads
        # alternate between the two HWDGE rings (SP and ACT sequencers).
        # x input is host-permuted tile-major: each tile's 1MB is contiguous
        # in DRAM (HBM row locality; one DMA instruction per tile).
        xts = []
        for t in range(NTILES):
            xt = xpool.tile([128, 4, 512], f32, tag="xt", name=f"xt{t}")
            nc.sync.dma_start(out=xt[:], in_=xin[t])
            xts.append(xt)

        for t in range(NTILES):
            xt = xts[t]
            for j in range(4):
                q = 4 * t + j
                ki = _KIDX[(q // 5, min(q + 128, 254) // 5)]
                if ki not in cs_tiles:
                    make_cs(ki)

            dt = dpool.tile([128, 4, W], f32, tag="dt")
            for j in range(4):
                q = 4 * t + j
                cst = get_cs(_KIDX[(q // 5, min(q + 128, 254) // 5)])
                # every 8th pair's subtract runs on GPSIMD to offload DVE
                eng = nc.gpsimd if q % 8 == 3 else nc.vector
                eng.tensor_tensor(
                    dt[:, j, :].rearrange("p (g f) -> p g f", f=5),
                    xt[:, j, 0:W].rearrange("p (g f) -> p g f", f=5),
                    cst[:, :, None].to_broadcast((128, NG, 5)),
                    OP.subtract,
                )

            d2t = d2pool.tile([128, 4, W], rdt, tag="d2t")
            nc.scalar.activation(d2t[:], dt[:], AF.Square,
                                 bias=zero_bias[:], scale=1.0)

            for j in range(4):
                q = 4 * t + j
                f = q // 64
                i2 = q % 64
                nc.tensor.matmul(
                    out=psum_tiles[f][:],
                    lhsT=PB[:, 126 - 2 * i2:254 - 2 * i2],
                    rhs=d2t[:, j, :],
                    start=(i2 == 0), stop=(i2 == 63),
                )

            if t == 15:
                pixel_finish(0)
            if t == NTILES - 1:
                pixel_finish(1)

        lsum = consts.tile([128, 1], f32, tag="lsum")
        nc.vector.tensor_tensor(lsum[:], lacc[0][:], lacc[1][:], OP.add)
        csum = consts.tile([128, 1], f32, tag="csum")
        nc.vector.tensor_tensor(csum[:], cacc[0][:], cacc[1][:], OP.add)
        nc.sync.dma_start(out=out[:, 0:1], in_=lsum[:])
        nc.sync.dma_start(out=out[:, 1:2], in_=csum[:])

    return nc


def get_module(split_waits=True):
    name = os.environ.get("DGP_REDUCE_DT", "float32r")
    key = (name, split_waits)
    if key not in _NC_CACHE:
        nc = _build_module(name)
        if split_waits:
            _split_excess_waits(nc)
        _NC_CACHE[key] = nc
    return _NC_CACHE[key]


def make_shards(seg_feat, dep_true):
    pb_s = np.zeros((128, 256), np.float32)
    pb_s[0:64, 126] = 1.0
    pb_s[64:128, 127] = 1.0
    rows_fp = np.empty((2, 128), np.int64)
    for f in range(2):
        for p in range(128):
            rows_fp[f, p] = _row_of(f, p)
    crow = np.minimum(5 * (rows_fp // 5) + 2, 254)

    shards = []
    for k in range(8):
        b, h = k // 2, k % 2
        seg_s = np.asarray(seg_feat[b, :, 255 * h:255 * h + 255, :],
                           dtype=np.float32)
        rowsA = np.arange(128).reshape(32, 4)
        rowsB = np.minimum(128 + np.arange(128), 254).reshape(32, 4)
        xa = seg_s[:, rowsA, :].transpose(1, 0, 2, 3)    # [32, 64, 4, 512]
        xb = seg_s[:, rowsB, :].transpose(1, 0, 2, 3)
        xin_s = np.ascontiguousarray(
            np.concatenate([xa[:, None], xb[:, None]], axis=1)
            .reshape(NTILES, 128, 4, 512))
        dep_s = np.ascontiguousarray(dep_true[b, 0, 255 * h:255 * h + 255, :],
                                     dtype=np.float32)
        dctr_s = np.ascontiguousarray(dep_s[crow][:, :, 2:512:5][:, :, :NG])
        dpx_s = np.ascontiguousarray(dep_s[rows_fp][:, :, :W])
        dpx_s[1, 127, :] = 0.0  # dup row excluded via the dep>eps mask
        cents = seg_s[:, 2::5, 2:512:5][:, :, :NG]       # [64, 51, 102]
        pcc0_s = np.empty((128, NG), np.float32)
        pcc0_s[0:64] = cents[:, 0]
        pcc0_s[64:128] = cents[:, 25]
        shards.append({
            "xin": xin_s,
            "dpx": dpx_s,
            "dctr": dctr_s,
            "pb": pb_s,
            "pcc0": pcc0_s,
        })
    return shards


def combine(outs):
    tl = 0.0
    tc_ = 0.0
    for o in outs:
        tl += float(np.sum(o["out"][:, 0].astype(np.float64)))
        tc_ += float(np.sum(o["out"][:, 1].astype(np.float64)))
    return np.float32(tl / max(tc_, 1.0))


def kernel(seg_feat, dep_true):
    from concourse.bass_utils import run_bass_kernel_spmd
    nc = get_module()
    shards = make_shards(np.asarray(seg_feat), np.asarray(dep_true))
    res = run_bass_kernel_spmd(nc, shards, list(range(8)))
    return combine(res.results)
